# revision 42
# baseline (speedup 1.0000x reference)
"""Sliding-window attention kernel for 8 TRN2 NeuronCores.

Sharding: core c owns heads {2c, 2c+1} for BOTH batches (tensor parallel
over the 16 heads).  After attention, an all-to-all redistributes the
per-head outputs so core c owns output rows (batch c//4, t-chunk c%4),
where it applies the full Wo projection.

Per-core pipeline (all matmul compute in bf16, fp32 accumulation):
  1. x (both batches) f32->bf16 cast DMA, then xbar-transpose load -> x^T
  2. Q^T/K^T/V^T projections (PE), RoPE via partition-swap DMA + DVE
  3. V^T -> V (natural layout) via DRAM bounce + transpose DMA, augmented
     with a ones column per head (gives softmax denominator for free)
  4. per (head, batch): banded scores S^T = K'Q'^T (PE) -> exp (ACT) ->
     boundary-triangle masks (DVE) -> E^T in SBUF
  5. AV: out^T[d,q] accumulated over k-tiles (PE), denominator = row 64
  6. all-to-all (bf16 numerators + denominators)
  7. receiver: reciprocal of denominators, broadcast scale, Wo matmul
"""
import numpy as np
import ml_dtypes

import concourse.bass as bass
import concourse.bacc as bacc
import concourse.mybir as mybir
import concourse.tile as tile

F32 = mybir.dt.float32
BF16 = mybir.dt.bfloat16
AF = mybir.ActivationFunctionType
ALU = mybir.AluOpType

B, T, D = 2, 2048, 1024
H, DH = 16, 64
WIN = T // 4              # 512
N_CORES = 8
HPC = H // N_CORES        # heads per core = 2
TC = T // 4               # output t-chunk per core = 512
KT = T // 128             # k-tiles per (head,batch) = 16
SCALE = 1.0 / np.sqrt(DH)

NKT = 128                 # k-tile rows
MAXW = 1152               # max window width per k-tile


def window(k0):
    """q-window [ws, we) for k-tile starting at k0."""
    return max(k0 - WIN, 0), min(k0 + NKT + WIN, T)


EOFF = []
_off = 0
for _kt in range(KT):
    _ws, _we = max(_kt*128 - WIN, 0), min(_kt*128 + NKT + WIN, T)
    EOFF.append(_off)
    _off += _we - _ws
ETOT = _off


def host_inputs(x, Wq, Wk, Wv, Wo, core):
    """Build the per-core input map (host-side shard + constant tables)."""
    bf = ml_dtypes.bfloat16
    cols = slice(core * HPC * DH, (core + 1) * HPC * DH)
    t = np.arange(T, dtype=np.float64)
    inv = 1.0 / (10000.0 ** (np.arange(0, DH, 2, dtype=np.float64) / DH))
    f = (t[:, None] * inv[None, :]).astype(np.float32)   # [T, 32]
    cos1 = np.cos(f).astype(np.float32)                  # [T, 32]
    sin1 = np.sin(f).astype(np.float32)
    # ^T layout tables [128, T]: row r -> head-dim d = r % 64
    d = np.arange(128) % 64
    cos_t = cos1.T[d % 32]                               # [128, T]
    sin_t = sin1.T[d % 32]
    sgn = np.where(d < 32, -1.0, 1.0).astype(np.float32)[:, None]
    sin_s = sin_t * sgn                                  # signed sin for swap trick
    kr = np.arange(128)[:, None]
    qc = np.arange(128)[None, :]
    tri_l = (kr <= qc).astype(np.float32)                # valid mask, left boundary
    tri_r = (qc <= kr).astype(np.float32)                # valid mask, right boundary
    return {
        "xt": np.ascontiguousarray(x.reshape(B * T, D).T.astype(bf)),
        "wq": np.ascontiguousarray(Wq[:, cols].astype(bf)),
        "wk": np.ascontiguousarray(Wk[:, cols].astype(bf)),
        "wv": np.ascontiguousarray(Wv[:, cols].astype(bf)),
        "wo": np.ascontiguousarray(Wo.astype(bf)),
        "cos_t": cos_t.astype(bf),
        "sin_s": sin_s.astype(bf),
        "tri_l": tri_l.astype(bf),
        "tri_r": tri_r.astype(bf),
    }


def host_assemble(results):
    """Concatenate the 8 per-core [512, 1024] chunks into [B, T, D]."""
    out = np.empty((B, T, D), np.float32)
    for c in range(N_CORES):
        out[c // 4, (c % 4) * TC:(c % 4 + 1) * TC, :] = results[c]["out"]
    return out


def build(nc, replicate=1, debug=False):
    x_d = nc.dram_tensor("xt", [D, B * T], BF16, kind="ExternalInput").ap()
    wq_d = nc.dram_tensor("wq", [D, HPC * DH], BF16, kind="ExternalInput").ap()
    wk_d = nc.dram_tensor("wk", [D, HPC * DH], BF16, kind="ExternalInput").ap()
    wv_d = nc.dram_tensor("wv", [D, HPC * DH], BF16, kind="ExternalInput").ap()
    wo_d = nc.dram_tensor("wo", [D, D], BF16, kind="ExternalInput").ap()
    cos_d = nc.dram_tensor("cos_t", [128, T], BF16, kind="ExternalInput").ap()
    sin_d = nc.dram_tensor("sin_s", [128, T], BF16, kind="ExternalInput").ap()
    tl_d = nc.dram_tensor("tri_l", [128, 128], BF16, kind="ExternalInput").ap()
    tr_d = nc.dram_tensor("tri_r", [128, 128], BF16, kind="ExternalInput").ap()
    out_d = nc.dram_tensor("out", [TC, D], F32, kind="ExternalOutput").ap()

    dbg = {}
    if debug:
        for name, shape, dt_ in [
            ("dbg_xt", [128, 8, 1024], BF16),
            ("dbg_q", [128, B, T], BF16),
            ("dbg_k", [128, B, T], BF16),
            ("dbg_vaug", [128, B * KT, 130], BF16),
            ("dbg_e", [128, ETOT], BF16),
            ("dbg_a", [65, T], BF16),
            ("dbg_at", [128, 8, TC], BF16),
            ("dbg_den", [16, TC], F32),
        ]:
            dbg[name] = nc.dram_tensor(name, shape, dt_, kind="ExternalOutput").ap()
    with tile.TileContext(nc) as tc:
        for _ in range(replicate):
            _build_once(nc, tc, x_d, wq_d, wk_d, wv_d, wo_d, cos_d, sin_d,
                        tl_d, tr_d, out_d, dbg)
    nc.compile()
    return nc


def _build_once(nc, tc, x_d, wq_d, wk_d, wv_d, wo_d, cos_d, sin_d, tl_d, tr_d,
                out_d, dbg={}):
    with tc.tile_pool(name="const", bufs=1) as constp, \
         tc.tile_pool(name="xt", bufs=2) as xtp, \
         tc.tile_pool(name="qk", bufs=1) as qkp, \
         tc.tile_pool(name="rope", bufs=2) as ropep, \
         tc.tile_pool(name="vt", bufs=2) as vtp, \
         tc.tile_pool(name="ebuf", bufs=2) as ep, \
         tc.tile_pool(name="abuf", bufs=1) as ap_, \
         tc.tile_pool(name="wop", bufs=1) as wop, \
         tc.tile_pool(name="fin", bufs=1) as finp, \
         tc.tile_pool(name="ps_small", bufs=2, space="PSUM") as pss, \
         tc.tile_pool(name="ps_big", bufs=2, space="PSUM") as psb, \
         tc.tile_pool(name="dram", bufs=1, space="DRAM") as dr:

        # ---------------- weights (bf16 from host) ----------------
        cos_sb = constp.tile([128, T], BF16, tag="cos")
        sin_sb = constp.tile([128, T], BF16, tag="sin")
        tl_sb = constp.tile([128, 128], BF16, tag="tl")
        tr_sb = constp.tile([128, 128], BF16, tag="tr")
        wq_sb = constp.tile([128, 8, HPC * DH], BF16, tag="wq")
        wk_sb = constp.tile([128, 8, HPC * DH], BF16, tag="wk")
        wv_sb = constp.tile([128, 8, HPC * DH], BF16, tag="wv")
        wo_sb = constp.tile([128, 8, D], BF16, tag="wo")
        # wq on the SP queue ahead of the first x quarter (so the first
        # projection matmuls start ASAP); wk/wv on the ACT queue; none of
        # them on Pool, which the gathers/staging keep busy.
        for cb in range(8):
            nc.sync.dma_start(wq_sb[:, cb, :], wq_d[cb * 128:(cb + 1) * 128, :])
        for cb in range(8):
            nc.scalar.dma_start(wk_sb[:, cb, :],
                                wk_d[cb * 128:(cb + 1) * 128, :])
            nc.scalar.dma_start(wv_sb[:, cb, :],
                                wv_d[cb * 128:(cb + 1) * 128, :])

        # ---------------- x^T (host pre-transposed bf16) ----------------
        NQ = 4                       # t-quarters of 1024 cols (over B*T=4096)
        QL = (B * T) // NQ           # 1024

        # Q^T/K^T/V^T projections, accumulating over c-blocks per quarter.
        # psum tile per (proj, t-chunk of 512): [128, 512]
        qraw = qkp.tile([128, B, T], BF16, tag="qp")     # becomes Q' after RoPE
        kraw = qkp.tile([128, B, T], BF16, tag="kp")
        v_aug = ap_.tile([128, B * KT, 130], BF16, tag="vaug")

        for q in range(NQ):
            xt = xtp.tile([128, 8, QL], BF16, tag="xt")
            for cb in range(8):
                nc.sync.dma_start(
                    xt[:, cb, :], x_d[cb * 128:(cb + 1) * 128,
                                      q * QL:(q + 1) * QL])
            for half in range(2):    # two 512-chunks per quarter
                t0 = q * QL + half * 512
                for pi, (wsb, dst) in enumerate(
                        ((wq_sb, qraw), (wk_sb, kraw))):
                    # alternate psum pools: ps_big is unused during projections
                    if (q * 6 + half * 3 + pi) % 2 == 0:
                        pt = pss.tile([128, 512], F32, tag="ps_small")
                    else:
                        pt = psb.tile([128, 512], F32, tag="ps_big",
                                      padded_shape=[128, MAXW])
                    for cb in range(8):
                        nc.tensor.matmul(
                            pt[:], wsb[:, cb, :],
                            xt[:, cb, half * 512:(half + 1) * 512],
                            start=(cb == 0), stop=(cb == 7))
                    b0, tt = divmod(t0, T)
                    nc.scalar.activation(dst[:, b0, tt:tt + 512], pt[:],
                                         AF.Copy)
                # V directly in natural layout (lhsT = x^T tile), written
                # straight into the 65-interleaved v_aug by DVE -- no DRAM
                # bounce, no transpose DMAs, no gather DMAs.
                for i in range(4):
                    ti = half * 4 + i
                    gb, gkt128 = divmod(t0 + i * 128, T)
                    gkt = gb * KT + gkt128 // 128
                    pv = pss.tile([128, 512], F32, tag="ps_small")
                    for cb in range(8):
                        nc.tensor.matmul(
                            pv[:, 0:128],
                            xt[:, cb, (half * 512 + i * 128) % 1024:
                               (half * 512 + i * 128) % 1024 + 128],
                            wv_sb[:, cb, :],
                            start=(cb == 0), stop=(cb == 7))
                    nc.vector.tensor_copy(v_aug[:, gkt, 0:64], pv[:, 0:64])
                    nc.vector.tensor_copy(v_aug[:, gkt, 65:129],
                                          pv[:, 64:128])

        nc.gpsimd.dma_start(cos_sb[:], cos_d[:])
        nc.gpsimd.dma_start(sin_sb[:], sin_d[:])
        nc.gpsimd.dma_start(tl_sb[:], tl_d[:])
        nc.gpsimd.dma_start(tr_sb[:], tr_d[:])
        for cb in range(8):
            nc.gpsimd.dma_start(wo_sb[:, cb, :], wo_d[cb * 128:(cb + 1) * 128, :])

        # ---------------- RoPE (on Q^T/K^T, per batch) ----------------
        for b in range(B):
            for dst in (qraw, kraw):
                sw = ropep.tile([128, T], BF16, tag="sw")
                # 32-block partition swap via SBUF->SBUF DMA
                for h2 in range(2):
                    p0 = h2 * 64
                    nc.sync.dma_start(sw[p0:p0 + 32, :],
                                      dst[p0 + 32:p0 + 64, b, :])
                    nc.sync.dma_start(sw[p0 + 32:p0 + 64, :],
                                      dst[p0:p0 + 32, b, :])
                nc.vector.tensor_mul(dst[:, b, :], dst[:, b, :], cos_sb[:])
                nc.vector.tensor_mul(sw[:], sw[:], sin_sb[:])
                nc.vector.tensor_add(dst[:, b, :], dst[:, b, :], sw[:])

        if "dbg_q" in dbg:
            nc.sync.dma_start(dbg["dbg_q"][:], qraw[:])
            nc.sync.dma_start(dbg["dbg_k"][:], kraw[:])

        one_view = v_aug[:].rearrange("p k (h e) -> p k h e", e=65)[:, :, :, 64]
        nc.vector.memset(one_view, 1.0)

        if "dbg_vaug" in dbg:
            nc.sync.dma_start(dbg["dbg_vaug"][:], v_aug[:])

        # ------------- all-to-all split by head-half (overlap) -------------
        # collective h2: chunk j -> core j = (batch j//4, t-chunk j%4);
        # rows 0-63 = A^T of the sender's head h2, row 64 = its denominator.
        a2a_in = [dr.tile([8, 65, TC], BF16, name=f"a2ai{i}", tag=f"a2ai{i}")
                  for i in range(2)]
        a2a_out = [dr.tile([8, 65, TC], BF16, name=f"a2ao{i}", tag=f"a2ao{i}")
                   for i in range(2)]
        at_sb = finp.tile([128, 8, TC], BF16, tag="at")
        den_sb = [finp.tile([8, TC], F32, name=f"den{i}", tag=f"den{i}")
                  for i in range(2)]
        rec_sb = [finp.tile([8, TC], F32, name=f"rec{i}", tag=f"rec{i}")
                  for i in range(2)]
        recs_sb = [vtp.tile([8, TC], F32, name=f"recs{i}", tag="rsc", bufs=3)
                   for i in range(2)]
        recb_sb = [finp.tile([8, TC], BF16, name=f"recb{i}", tag=f"recb{i}")
                   for i in range(2)]
        rec_dr = [dr.tile([8, TC], BF16, name=f"recdr{i}", tag=f"recdr{i}")
                  for i in range(2)]

        def emit_a2a(h2):
            for j in range(8):
                jb, jt = j // 4, j % 4
                cols = slice(jt * TC, (jt + 1) * TC)
                nc.gpsimd.dma_start(a2a_in[h2][j, 0:65, :],
                                  a_out[h2 * B + jb][0:65, cols])
            nc.gpsimd.collective_compute(
                "AllToAll", ALU.bypass, replica_groups=[list(range(N_CORES))],
                ins=[a2a_in[h2].opt()], outs=[a2a_out[h2].opt()])

        def recv_a2a(h2):
            # den row c = head h2 of sender c; at rows 64*h2.. per c-block
            nc.gpsimd.dma_start(den_sb[h2][:], a2a_out[h2][:, 64, :])
            nc.vector.reciprocal_approx_accurate(
                out=rec_sb[h2][:], in_=den_sb[h2][:], scratch=recs_sb[h2][:])
            nc.vector.tensor_copy(recb_sb[h2][:], rec_sb[h2][:])
            nc.sync.dma_start(rec_dr[h2][:], recb_sb[h2][:])
            for c in range(8):
                nc.sync.dma_start(at_sb[64 * h2:64 * h2 + 64, c, :],
                                  a2a_out[h2][c, 0:64, :])
                r_sc = vtp.tile([128, TC], BF16, tag="rsc", bufs=3)
                p0 = 64 * h2
                nc.sync.dma_start(
                    r_sc[p0:p0 + 64, :],
                    rec_dr[h2][c:c + 1, :].to_broadcast((64, TC)))
                nc.vector.tensor_mul(at_sb[p0:p0 + 64, c, :],
                                     at_sb[p0:p0 + 64, c, :], r_sc[p0:p0 + 64, :])

        # -------- rearrange Q'/K' to batch-on-partition-halves per head ----
        # Qd/Kd[h]: rows 0-63 = (head h, batch 0), rows 64-127 = (h, batch 1)
        # so the two batches' 64-deep score matmuls row-pack in the PE array.
        qd = {}
        kd = {}
        for h2 in range(2):
            qd[h2] = xtp.tile([128, T], BF16, name=f"qd{h2}", tag="xt")
            kd[h2] = xtp.tile([128, T], BF16, name=f"kd{h2}", tag="xt")
            for b in range(B):
                nc.sync.dma_start(qd[h2][64 * b:64 * b + 64, :],
                                  qraw[64 * h2:64 * h2 + 64, b, :])
                nc.sync.dma_start(kd[h2][64 * b:64 * b + 64, :],
                                  kraw[64 * h2:64 * h2 + 64, b, :])

        # ---------------- attention: head-major batch-paired phases --------
        a_out = [None] * 4   # [65, T] numerator^T (+den row 64) per (h2, b)
        for h2 in range(2):
            e_sb = {}
            for b in range(B):
                a_out[h2 * B + b] = ap_.tile([65, T], BF16, name=f"a{h2}{b}",
                                             tag=f"a{h2}{b}")
                e_sb[b] = ep.tile([128, ETOT], BF16, name=f"e{h2}{b}", tag="E")

            def emit_av(b, qc, h2=h2, e_sb=e_sb):
                q0 = qc * 512
                kts = [kt for kt in range(KT)
                       if window(kt * 128)[0] < q0 + 512
                       and window(kt * 128)[1] > q0]
                av = pss.tile([65, 512], F32, tag="ps_small")
                for i, kt in enumerate(kts):
                    ws, we = window(kt * 128)
                    lo = max(q0, ws)
                    hi = min(q0 + 512, we)
                    nc.tensor.matmul(
                        av[:, lo - q0:hi - q0],
                        v_aug[:, b * KT + kt, 65 * h2:65 * h2 + 65],
                        e_sb[b][:, EOFF[kt] + lo - ws:EOFF[kt] + hi - ws],
                        start=(i == 0), stop=(i == len(kts) - 1))
                nc.vector.tensor_copy(a_out[h2 * B + b][:, q0:q0 + 512], av[:])

            for kt in range(KT):
                k0 = kt * 128
                ws, we = window(k0)
                W = we - ws
                sc = {}
                for b in range(B):    # adjacent 64-deep MMs -> row-packed
                    p0 = 64 * b
                    sc[b] = psb.tile([128, MAXW], F32, name=f"sc{b}",
                                     tag="ps_big")
                    lhsT = kd[h2][p0:p0 + 64, k0:k0 + 128]
                    for s0 in range(0, W, 512):
                        s1 = min(s0 + 512, W)
                        nc.tensor.matmul(sc[b][:, s0:s1],
                                         lhsT,
                                         qd[h2][p0:p0 + 64, ws + s0:ws + s1],
                                         start=True, stop=True)
                eo = EOFF[kt]
                for b in range(B):
                    nc.scalar.activation(e_sb[b][:, eo:eo + W], sc[b][:, 0:W],
                                         AF.Exp, scale=SCALE)
                    if k0 >= WIN:
                        nc.vector.tensor_mul(e_sb[b][:, eo:eo + 128],
                                             e_sb[b][:, eo:eo + 128], tl_sb[:])
                    if k0 + 128 + WIN <= T:
                        nc.vector.tensor_mul(
                            e_sb[b][:, eo + W - 128:eo + W],
                            e_sb[b][:, eo + W - 128:eo + W], tr_sb[:])
                    for qc in range(4):
                        if min(4 * qc + 7, KT - 1) == kt:
                            emit_av(b, qc)
            if h2 == 0 and "dbg_e" in dbg:
                nc.sync.dma_start(dbg["dbg_e"][:], e_sb[0][:])
            emit_a2a(h2)

        if "dbg_a" in dbg:
            nc.sync.dma_start(dbg["dbg_a"][:], a_out[0][:])

        recv_a2a(0)
        recv_a2a(1)
        if "dbg_at" in dbg:
            nc.sync.dma_start(dbg["dbg_at"][:], at_sb[:])
            nc.sync.dma_start(dbg["dbg_den"][:], den_sb[:])
        # Wo split by head-half: the h2=0 half of the contraction only needs
        # the first collective, so it overlaps the second one; halves are
        # combined with an accumulating DMA into the output.
        for h2 in range(2):
            p0 = 64 * h2
            for tt in range(4):
                for mh in range(2):
                    po = pss.tile([128, 512], F32, tag="ps_small")
                    for c in range(8):
                        nc.tensor.matmul(
                            po[:],
                            at_sb[p0:p0 + 64, c, tt * 128:(tt + 1) * 128],
                            wo_sb[p0:p0 + 64, c, mh * 512:(mh + 1) * 512],
                            start=(c == 0), stop=(c == 7))
                    ot = vtp.tile([128, 512], F32, tag="ot")
                    nc.vector.tensor_copy(ot[:], po[:])
                    dst = out_d[tt * 128:(tt + 1) * 128,
                                mh * 512:(mh + 1) * 512]
                    if h2 == 0:
                        nc.sync.dma_start(dst, ot[:])
                    else:
                        nc.gpsimd.dma_start(dst, ot[:], accum_op=ALU.add)


# ---------------------------------------------------------------------------
# Self-contained entry point: kernel(**inputs) -> full output [2, 2048, 1024]
# ---------------------------------------------------------------------------
_CACHE = {}


def _get_nc():
    if "nc" in _CACHE:
        return _CACHE["nc"]
    import concourse.bacc as _bacc
    nc = _bacc.Bacc("TRN2", target_bir_lowering=False, debug=False,
                    num_devices=N_CORES)
    build(nc)
    _CACHE["nc"] = nc
    return nc


def kernel(x, Wq, Wk, Wv, Wo):
    from concourse.bass_utils import run_bass_kernel_spmd
    x, Wq, Wk, Wv, Wo = (np.asarray(a, np.float32) for a in (x, Wq, Wk, Wv, Wo))
    nc = _get_nc()
    in_maps = [host_inputs(x, Wq, Wk, Wv, Wo, c) for c in range(N_CORES)]
    res = run_bass_kernel_spmd(nc, in_maps, core_ids=list(range(N_CORES)))
    return host_assemble(res.results)



# revision 43
# speedup vs baseline: 1.0613x; 1.0613x over previous
"""Sliding-window attention kernel for 8 TRN2 NeuronCores.

Sharding: core c owns heads {2c, 2c+1} for BOTH batches (tensor parallel
over the 16 heads).  After attention, an all-to-all redistributes the
per-head outputs so core c owns output rows (batch c//4, t-chunk c%4),
where it applies the full Wo projection.

Per-core pipeline (all matmul compute in bf16, fp32 accumulation):
  1. x (both batches) f32->bf16 cast DMA, then xbar-transpose load -> x^T
  2. Q^T/K^T/V^T projections (PE), RoPE via partition-swap DMA + DVE
  3. V^T -> V (natural layout) via DRAM bounce + transpose DMA, augmented
     with a ones column per head (gives softmax denominator for free)
  4. per (head, batch): banded scores S^T = K'Q'^T (PE) -> exp (ACT) ->
     boundary-triangle masks (DVE) -> E^T in SBUF
  5. AV: out^T[d,q] accumulated over k-tiles (PE), denominator = row 64
  6. all-to-all (bf16 numerators + denominators)
  7. receiver: reciprocal of denominators, broadcast scale, Wo matmul
"""
import numpy as np
import ml_dtypes

import concourse.bass as bass
import concourse.bacc as bacc
import concourse.mybir as mybir
import concourse.tile as tile

F32 = mybir.dt.float32
BF16 = mybir.dt.bfloat16
AF = mybir.ActivationFunctionType
ALU = mybir.AluOpType

B, T, D = 2, 2048, 1024
H, DH = 16, 64
WIN = T // 4              # 512
N_CORES = 8
HPC = H // N_CORES        # heads per core = 2
TC = T // 4               # output t-chunk per core = 512
KT = T // 128             # k-tiles per (head,batch) = 16
SCALE = 1.0 / np.sqrt(DH)

NKT = 128                 # k-tile rows
MAXW = 1152               # max window width per k-tile


def window(k0):
    """q-window [ws, we) for k-tile starting at k0."""
    return max(k0 - WIN, 0), min(k0 + NKT + WIN, T)


EOFF = []
_off = 0
for _kt in range(KT):
    _ws, _we = max(_kt*128 - WIN, 0), min(_kt*128 + NKT + WIN, T)
    EOFF.append(_off)
    _off += _we - _ws
ETOT = _off


def host_inputs(x, Wq, Wk, Wv, Wo, core):
    """Build the per-core input map (host-side shard + constant tables)."""
    bf = ml_dtypes.bfloat16
    cols = slice(core * HPC * DH, (core + 1) * HPC * DH)
    t = np.arange(T, dtype=np.float64)
    inv = 1.0 / (10000.0 ** (np.arange(0, DH, 2, dtype=np.float64) / DH))
    f = (t[:, None] * inv[None, :]).astype(np.float32)   # [T, 32]
    cos1 = np.cos(f).astype(np.float32)                  # [T, 32]
    sin1 = np.sin(f).astype(np.float32)
    # ^T layout tables [128, T]: row r -> head-dim d = r % 64
    d = np.arange(128) % 64
    cos_t = cos1.T[d % 32]                               # [128, T]
    sin_t = sin1.T[d % 32]
    sgn = np.where(d < 32, -1.0, 1.0).astype(np.float32)[:, None]
    sin_s = sin_t * sgn                                  # signed sin for swap trick
    kr = np.arange(128)[:, None]
    qc = np.arange(128)[None, :]
    tri_l = (kr <= qc).astype(np.float32)                # valid mask, left boundary
    tri_r = (qc <= kr).astype(np.float32)                # valid mask, right boundary
    return {
        "xt": np.ascontiguousarray(x.reshape(B * T, D).T.astype(bf)),
        "wq": np.ascontiguousarray(Wq[:, cols].astype(bf)),
        "wk": np.ascontiguousarray(Wk[:, cols].astype(bf)),
        "wv": np.ascontiguousarray(Wv[:, cols].astype(bf)),
        "wo": np.ascontiguousarray(Wo.astype(bf)),
        "cos_t": cos_t.astype(bf),
        "sin_s": sin_s.astype(bf),
        "tri_l": tri_l.astype(bf),
        "tri_r": tri_r.astype(bf),
    }


def host_assemble(results):
    """Concatenate the 8 per-core [512, 1024] chunks into [B, T, D]."""
    out = np.empty((B, T, D), np.float32)
    for c in range(N_CORES):
        out[c // 4, (c % 4) * TC:(c % 4 + 1) * TC, :] = results[c]["out"]
    return out


def build(nc, replicate=1, debug=False):
    x_d = nc.dram_tensor("xt", [D, B * T], BF16, kind="ExternalInput").ap()
    wq_d = nc.dram_tensor("wq", [D, HPC * DH], BF16, kind="ExternalInput").ap()
    wk_d = nc.dram_tensor("wk", [D, HPC * DH], BF16, kind="ExternalInput").ap()
    wv_d = nc.dram_tensor("wv", [D, HPC * DH], BF16, kind="ExternalInput").ap()
    wo_d = nc.dram_tensor("wo", [D, D], BF16, kind="ExternalInput").ap()
    cos_d = nc.dram_tensor("cos_t", [128, T], BF16, kind="ExternalInput").ap()
    sin_d = nc.dram_tensor("sin_s", [128, T], BF16, kind="ExternalInput").ap()
    tl_d = nc.dram_tensor("tri_l", [128, 128], BF16, kind="ExternalInput").ap()
    tr_d = nc.dram_tensor("tri_r", [128, 128], BF16, kind="ExternalInput").ap()
    out_d = nc.dram_tensor("out", [TC, D], F32, kind="ExternalOutput").ap()

    dbg = {}
    if debug:
        for name, shape, dt_ in [
            ("dbg_xt", [128, 8, 1024], BF16),
            ("dbg_q", [128, B, T], BF16),
            ("dbg_k", [128, B, T], BF16),
            ("dbg_vaug", [128, B * KT, 130], BF16),
            ("dbg_e", [128, ETOT], BF16),
            ("dbg_a", [65, T], BF16),
            ("dbg_at", [128, 8, TC], BF16),
            ("dbg_den", [16, TC], F32),
        ]:
            dbg[name] = nc.dram_tensor(name, shape, dt_, kind="ExternalOutput").ap()
    with tile.TileContext(nc) as tc:
        for _ in range(replicate):
            _build_once(nc, tc, x_d, wq_d, wk_d, wv_d, wo_d, cos_d, sin_d,
                        tl_d, tr_d, out_d, dbg)
    nc.compile()
    return nc


def _build_once(nc, tc, x_d, wq_d, wk_d, wv_d, wo_d, cos_d, sin_d, tl_d, tr_d,
                out_d, dbg={}):
    with tc.tile_pool(name="const", bufs=1) as constp, \
         tc.tile_pool(name="xt", bufs=2) as xtp, \
         tc.tile_pool(name="qk", bufs=1) as qkp, \
         tc.tile_pool(name="rope", bufs=2) as ropep, \
         tc.tile_pool(name="vt", bufs=2) as vtp, \
         tc.tile_pool(name="ebuf", bufs=2) as ep, \
         tc.tile_pool(name="abuf", bufs=1) as ap_, \
         tc.tile_pool(name="wop", bufs=1) as wop, \
         tc.tile_pool(name="fin", bufs=1) as finp, \
         tc.tile_pool(name="ps_small", bufs=2, space="PSUM") as pss, \
         tc.tile_pool(name="ps_big", bufs=2, space="PSUM") as psb, \
         tc.tile_pool(name="dram", bufs=1, space="DRAM") as dr:

        # ---------------- weights (bf16 from host) ----------------
        cos_sb = constp.tile([128, T], BF16, tag="cos")
        sin_sb = constp.tile([128, T], BF16, tag="sin")
        tl_sb = constp.tile([128, 128], BF16, tag="tl")
        tr_sb = constp.tile([128, 128], BF16, tag="tr")
        wq_sb = constp.tile([128, 8, HPC * DH], BF16, tag="wq")
        wk_sb = constp.tile([128, 8, HPC * DH], BF16, tag="wk")
        wv_sb = constp.tile([128, 8, HPC * DH], BF16, tag="wv")
        wo_sb = constp.tile([128, 8, D], BF16, tag="wo")
        # wq on the SP queue ahead of the first x quarter (so the first
        # projection matmuls start ASAP); wk/wv on the ACT queue; none of
        # them on Pool, which the gathers/staging keep busy.
        for cb in range(8):
            nc.sync.dma_start(wq_sb[:, cb, :], wq_d[cb * 128:(cb + 1) * 128, :])
        for cb in range(8):
            nc.scalar.dma_start(wk_sb[:, cb, :],
                                wk_d[cb * 128:(cb + 1) * 128, :])
            nc.scalar.dma_start(wv_sb[:, cb, :],
                                wv_d[cb * 128:(cb + 1) * 128, :])

        # ---------------- x^T (host pre-transposed bf16) ----------------
        NQ = 4                       # t-quarters of 1024 cols (over B*T=4096)
        QL = (B * T) // NQ           # 1024

        # Q^T/K^T/V^T projections, accumulating over c-blocks per quarter.
        # psum tile per (proj, t-chunk of 512): [128, 512]
        qraw = qkp.tile([128, B, T], BF16, tag="qp")     # becomes Q' after RoPE
        kraw = qkp.tile([128, B, T], BF16, tag="kp")
        v_aug = ap_.tile([128, B * KT, 130], BF16, tag="vaug")

        for q in range(NQ):
            xt = xtp.tile([128, 8, QL], BF16, tag="xt")
            for cb in range(8):
                nc.sync.dma_start(
                    xt[:, cb, :], x_d[cb * 128:(cb + 1) * 128,
                                      q * QL:(q + 1) * QL])
            for half in range(2):    # two 512-chunks per quarter
                t0 = q * QL + half * 512
                for pi, (wsb, dst) in enumerate(
                        ((wq_sb, qraw), (wk_sb, kraw))):
                    # alternate psum pools: ps_big is unused during projections
                    if (q * 6 + half * 3 + pi) % 2 == 0:
                        pt = pss.tile([128, 512], F32, tag="ps_small")
                    else:
                        pt = psb.tile([128, 512], F32, tag="ps_big",
                                      padded_shape=[128, MAXW])
                    for cb in range(8):
                        nc.tensor.matmul(
                            pt[:], wsb[:, cb, :],
                            xt[:, cb, half * 512:(half + 1) * 512],
                            start=(cb == 0), stop=(cb == 7))
                    b0, tt = divmod(t0, T)
                    nc.scalar.activation(dst[:, b0, tt:tt + 512], pt[:],
                                         AF.Copy)
                # V directly in natural layout (lhsT = x^T tile), written
                # straight into the 65-interleaved v_aug by DVE -- no DRAM
                # bounce, no transpose DMAs, no gather DMAs.
                for i in range(4):
                    ti = half * 4 + i
                    gb, gkt128 = divmod(t0 + i * 128, T)
                    gkt = gb * KT + gkt128 // 128
                    pv = pss.tile([128, 512], F32, tag="ps_small")
                    for cb in range(8):
                        nc.tensor.matmul(
                            pv[:, 0:128],
                            xt[:, cb, (half * 512 + i * 128) % 1024:
                               (half * 512 + i * 128) % 1024 + 128],
                            wv_sb[:, cb, :],
                            start=(cb == 0), stop=(cb == 7))
                    nc.vector.tensor_copy(v_aug[:, gkt, 0:64], pv[:, 0:64])
                    nc.vector.tensor_copy(v_aug[:, gkt, 65:129],
                                          pv[:, 64:128])

        nc.gpsimd.dma_start(cos_sb[:], cos_d[:])
        nc.gpsimd.dma_start(sin_sb[:], sin_d[:])
        nc.gpsimd.dma_start(tl_sb[:], tl_d[:])
        nc.gpsimd.dma_start(tr_sb[:], tr_d[:])
        for cb in range(8):
            nc.gpsimd.dma_start(wo_sb[:, cb, :], wo_d[cb * 128:(cb + 1) * 128, :])

        # ---------------- RoPE (on Q^T/K^T, per batch) ----------------
        for b in range(B):
            for dst in (qraw, kraw):
                sw = ropep.tile([128, T], BF16, tag="sw")
                # 32-block partition swap via SBUF->SBUF DMA
                for h2 in range(2):
                    p0 = h2 * 64
                    nc.sync.dma_start(sw[p0:p0 + 32, :],
                                      dst[p0 + 32:p0 + 64, b, :])
                    nc.sync.dma_start(sw[p0 + 32:p0 + 64, :],
                                      dst[p0:p0 + 32, b, :])
                nc.vector.tensor_mul(dst[:, b, :], dst[:, b, :], cos_sb[:])
                nc.vector.tensor_mul(sw[:], sw[:], sin_sb[:])
                nc.vector.tensor_add(dst[:, b, :], dst[:, b, :], sw[:])

        if "dbg_q" in dbg:
            nc.sync.dma_start(dbg["dbg_q"][:], qraw[:])
            nc.sync.dma_start(dbg["dbg_k"][:], kraw[:])

        one_view = v_aug[:].rearrange("p k (h e) -> p k h e", e=65)[:, :, :, 64]
        nc.vector.memset(one_view, 1.0)

        if "dbg_vaug" in dbg:
            nc.sync.dma_start(dbg["dbg_vaug"][:], v_aug[:])

        # ------------- all-to-all split by head-half (overlap) -------------
        # collective h2: chunk j -> core j = (batch j//4, t-chunk j%4);
        # rows 0-63 = A^T of the sender's head h2, row 64 = its denominator.
        a2a_in = [dr.tile([8, 65, TC], BF16, name=f"a2ai{i}", tag=f"a2ai{i}")
                  for i in range(2)]
        a2a_out = [dr.tile([8, 65, TC], BF16, name=f"a2ao{i}", tag=f"a2ao{i}")
                   for i in range(2)]
        at_sb = finp.tile([128, 8, TC], BF16, tag="at")
        den_sb = [finp.tile([8, TC], F32, name=f"den{i}", tag=f"den{i}")
                  for i in range(2)]
        rec_sb = [finp.tile([8, TC], F32, name=f"rec{i}", tag=f"rec{i}")
                  for i in range(2)]
        recs_sb = [vtp.tile([8, TC], F32, name=f"recs{i}", tag="rsc", bufs=3)
                   for i in range(2)]
        recb_sb = [finp.tile([8, TC], BF16, name=f"recb{i}", tag=f"recb{i}")
                   for i in range(2)]
        rec_dr = [dr.tile([8, TC], BF16, name=f"recdr{i}", tag=f"recdr{i}")
                  for i in range(2)]

        def emit_a2a(h2):
            for j in range(8):
                jb, jt = j // 4, j % 4
                cols = slice(jt * TC, (jt + 1) * TC)
                nc.gpsimd.dma_start(a2a_in[h2][j, 0:65, :],
                                  a_out[h2 * B + jb][0:65, cols])
            nc.gpsimd.collective_compute(
                "AllToAll", ALU.bypass, replica_groups=[list(range(N_CORES))],
                ins=[a2a_in[h2].opt()], outs=[a2a_out[h2].opt()])

        last_dve = {}

        def recv_a2a(h2):
            # den row c = head h2 of sender c; at rows 64*h2.. per c-block.
            # Every DVE op here is PINNED after the last attention-phase DVE
            # op: the scheduler otherwise hoists the reciprocal (which waits
            # on the collective-gated den DMA) into the middle of the h2=1
            # DVE stream, head-of-line blocking the queue for ~13 us.
            from concourse.tile_rust import add_dep_helper
            from concourse.dve_ops import RECIPROCAL_APPROX_NR
            anchor = last_dve[1].ins if 1 in last_dve else None

            def pin(bi):
                if anchor is not None:
                    add_dep_helper(bi.ins, anchor,
                                   sync=False,
                                   reason="keep recv chain behind attention "
                                          "DVE stream")
                return bi

            nc.gpsimd.dma_start(den_sb[h2][:], a2a_out[h2][:, 64, :])
            pin(nc.vector.reciprocal_approx_fast(out=recs_sb[h2][:],
                                                 in_=den_sb[h2][:]))
            pin(nc.vector._custom_dve(RECIPROCAL_APPROX_NR, out=rec_sb[h2][:],
                                      in0=den_sb[h2][:], in1=recs_sb[h2][:],
                                      s0=2.0))
            pin(nc.vector.tensor_copy(recb_sb[h2][:], rec_sb[h2][:]))
            nc.sync.dma_start(rec_dr[h2][:], recb_sb[h2][:])
            for c in range(8):
                nc.sync.dma_start(at_sb[64 * h2:64 * h2 + 64, c, :],
                                  a2a_out[h2][c, 0:64, :])
                r_sc = vtp.tile([128, TC], BF16, tag="rsc", bufs=3)
                p0 = 64 * h2
                nc.sync.dma_start(
                    r_sc[p0:p0 + 64, :],
                    rec_dr[h2][c:c + 1, :].to_broadcast((64, TC)))
                pin(nc.vector.tensor_mul(at_sb[p0:p0 + 64, c, :],
                                         at_sb[p0:p0 + 64, c, :],
                                         r_sc[p0:p0 + 64, :]))

        # -------- rearrange Q'/K' to batch-on-partition-halves per head ----
        # Qd/Kd[h]: rows 0-63 = (head h, batch 0), rows 64-127 = (h, batch 1)
        # so the two batches' 64-deep score matmuls row-pack in the PE array.
        qd = {}
        kd = {}
        for h2 in range(2):
            qd[h2] = xtp.tile([128, T], BF16, name=f"qd{h2}", tag="xt")
            kd[h2] = xtp.tile([128, T], BF16, name=f"kd{h2}", tag="xt")
            for b in range(B):
                nc.sync.dma_start(qd[h2][64 * b:64 * b + 64, :],
                                  qraw[64 * h2:64 * h2 + 64, b, :])
                nc.sync.dma_start(kd[h2][64 * b:64 * b + 64, :],
                                  kraw[64 * h2:64 * h2 + 64, b, :])

        # ---------------- attention: head-major batch-paired phases --------
        a_out = [None] * 4   # [65, T] numerator^T (+den row 64) per (h2, b)
        for h2 in range(2):
            e_sb = {}
            for b in range(B):
                a_out[h2 * B + b] = ap_.tile([65, T], BF16, name=f"a{h2}{b}",
                                             tag=f"a{h2}{b}")
                e_sb[b] = ep.tile([128, ETOT], BF16, name=f"e{h2}{b}", tag="E")

            def emit_av(b, qc, h2=h2, e_sb=e_sb):
                q0 = qc * 512
                kts = [kt for kt in range(KT)
                       if window(kt * 128)[0] < q0 + 512
                       and window(kt * 128)[1] > q0]
                av = pss.tile([65, 512], F32, tag="ps_small")
                for i, kt in enumerate(kts):
                    ws, we = window(kt * 128)
                    lo = max(q0, ws)
                    hi = min(q0 + 512, we)
                    nc.tensor.matmul(
                        av[:, lo - q0:hi - q0],
                        v_aug[:, b * KT + kt, 65 * h2:65 * h2 + 65],
                        e_sb[b][:, EOFF[kt] + lo - ws:EOFF[kt] + hi - ws],
                        start=(i == 0), stop=(i == len(kts) - 1))
                last_dve[h2] = nc.vector.tensor_copy(
                    a_out[h2 * B + b][:, q0:q0 + 512], av[:])

            for kt in range(KT):
                k0 = kt * 128
                ws, we = window(k0)
                W = we - ws
                sc = {}
                for b in range(B):    # adjacent 64-deep MMs -> row-packed
                    p0 = 64 * b
                    sc[b] = psb.tile([128, MAXW], F32, name=f"sc{b}",
                                     tag="ps_big")
                    lhsT = kd[h2][p0:p0 + 64, k0:k0 + 128]
                    for s0 in range(0, W, 512):
                        s1 = min(s0 + 512, W)
                        nc.tensor.matmul(sc[b][:, s0:s1],
                                         lhsT,
                                         qd[h2][p0:p0 + 64, ws + s0:ws + s1],
                                         start=True, stop=True)
                eo = EOFF[kt]
                for b in range(B):
                    nc.scalar.activation(e_sb[b][:, eo:eo + W], sc[b][:, 0:W],
                                         AF.Exp, scale=SCALE)
                    if k0 >= WIN:
                        nc.vector.tensor_mul(e_sb[b][:, eo:eo + 128],
                                             e_sb[b][:, eo:eo + 128], tl_sb[:])
                    if k0 + 128 + WIN <= T:
                        nc.vector.tensor_mul(
                            e_sb[b][:, eo + W - 128:eo + W],
                            e_sb[b][:, eo + W - 128:eo + W], tr_sb[:])
                    for qc in range(4):
                        if min(4 * qc + 7, KT - 1) == kt:
                            emit_av(b, qc)
            if h2 == 0 and "dbg_e" in dbg:
                nc.sync.dma_start(dbg["dbg_e"][:], e_sb[0][:])
            emit_a2a(h2)

        if "dbg_a" in dbg:
            nc.sync.dma_start(dbg["dbg_a"][:], a_out[0][:])

        recv_a2a(0)
        recv_a2a(1)
        if "dbg_at" in dbg:
            nc.sync.dma_start(dbg["dbg_at"][:], at_sb[:])
            nc.sync.dma_start(dbg["dbg_den"][:], den_sb[:])
        # Wo split by head-half: the h2=0 half of the contraction only needs
        # the first collective, so it overlaps the second one; halves are
        # combined with an accumulating DMA into the output.
        for h2 in range(2):
            p0 = 64 * h2
            for tt in range(4):
                for mh in range(2):
                    po = pss.tile([128, 512], F32, tag="ps_small")
                    for c in range(8):
                        nc.tensor.matmul(
                            po[:],
                            at_sb[p0:p0 + 64, c, tt * 128:(tt + 1) * 128],
                            wo_sb[p0:p0 + 64, c, mh * 512:(mh + 1) * 512],
                            start=(c == 0), stop=(c == 7))
                    ot = vtp.tile([128, 512], F32, tag="ot")
                    nc.vector.tensor_copy(ot[:], po[:])
                    dst = out_d[tt * 128:(tt + 1) * 128,
                                mh * 512:(mh + 1) * 512]
                    if h2 == 0:
                        nc.sync.dma_start(dst, ot[:])
                    else:
                        nc.gpsimd.dma_start(dst, ot[:], accum_op=ALU.add)


# ---------------------------------------------------------------------------
# Self-contained entry point: kernel(**inputs) -> full output [2, 2048, 1024]
# ---------------------------------------------------------------------------
_CACHE = {}


def _get_nc():
    if "nc" in _CACHE:
        return _CACHE["nc"]
    import concourse.bacc as _bacc
    nc = _bacc.Bacc("TRN2", target_bir_lowering=False, debug=False,
                    num_devices=N_CORES)
    build(nc)
    _CACHE["nc"] = nc
    return nc


def kernel(x, Wq, Wk, Wv, Wo):
    from concourse.bass_utils import run_bass_kernel_spmd
    x, Wq, Wk, Wv, Wo = (np.asarray(a, np.float32) for a in (x, Wq, Wk, Wv, Wo))
    nc = _get_nc()
    in_maps = [host_inputs(x, Wq, Wk, Wv, Wo, c) for c in range(N_CORES)]
    res = run_bass_kernel_spmd(nc, in_maps, core_ids=list(range(N_CORES)))
    return host_assemble(res.results)



# revision 44
# speedup vs baseline: 1.0704x; 1.0086x over previous
"""Sliding-window attention kernel for 8 TRN2 NeuronCores.

Sharding: core c owns heads {2c, 2c+1} for BOTH batches (tensor parallel
over the 16 heads).  After attention, an all-to-all redistributes the
per-head outputs so core c owns output rows (batch c//4, t-chunk c%4),
where it applies the full Wo projection.

Per-core pipeline (all matmul compute in bf16, fp32 accumulation):
  1. x (both batches) f32->bf16 cast DMA, then xbar-transpose load -> x^T
  2. Q^T/K^T/V^T projections (PE), RoPE via partition-swap DMA + DVE
  3. V^T -> V (natural layout) via DRAM bounce + transpose DMA, augmented
     with a ones column per head (gives softmax denominator for free)
  4. per (head, batch): banded scores S^T = K'Q'^T (PE) -> exp (ACT) ->
     boundary-triangle masks (DVE) -> E^T in SBUF
  5. AV: out^T[d,q] accumulated over k-tiles (PE), denominator = row 64
  6. all-to-all (bf16 numerators + denominators)
  7. receiver: reciprocal of denominators, broadcast scale, Wo matmul
"""
import numpy as np
import ml_dtypes

import concourse.bass as bass
import concourse.bacc as bacc
import concourse.mybir as mybir
import concourse.tile as tile

F32 = mybir.dt.float32
BF16 = mybir.dt.bfloat16
AF = mybir.ActivationFunctionType
ALU = mybir.AluOpType

B, T, D = 2, 2048, 1024
H, DH = 16, 64
WIN = T // 4              # 512
N_CORES = 8
HPC = H // N_CORES        # heads per core = 2
TC = T // 4               # output t-chunk per core = 512
KT = T // 128             # k-tiles per (head,batch) = 16
SCALE = 1.0 / np.sqrt(DH)

NKT = 128                 # k-tile rows
MAXW = 1152               # max window width per k-tile


def window(k0):
    """q-window [ws, we) for k-tile starting at k0."""
    return max(k0 - WIN, 0), min(k0 + NKT + WIN, T)


EOFF = []
_off = 0
for _kt in range(KT):
    _ws, _we = max(_kt*128 - WIN, 0), min(_kt*128 + NKT + WIN, T)
    EOFF.append(_off)
    _off += _we - _ws
ETOT = _off


def host_inputs(x, Wq, Wk, Wv, Wo, core):
    """Build the per-core input map (host-side shard + constant tables)."""
    bf = ml_dtypes.bfloat16
    cols = slice(core * HPC * DH, (core + 1) * HPC * DH)
    t = np.arange(T, dtype=np.float64)
    inv = 1.0 / (10000.0 ** (np.arange(0, DH, 2, dtype=np.float64) / DH))
    f = (t[:, None] * inv[None, :]).astype(np.float32)   # [T, 32]
    cos1 = np.cos(f).astype(np.float32)                  # [T, 32]
    sin1 = np.sin(f).astype(np.float32)
    # ^T layout tables [128, T]: row r -> head-dim d = r % 64
    d = np.arange(128) % 64
    cos_t = cos1.T[d % 32]                               # [128, T]
    sin_t = sin1.T[d % 32]
    sgn = np.where(d < 32, -1.0, 1.0).astype(np.float32)[:, None]
    sin_s = sin_t * sgn                                  # signed sin for swap trick
    kr = np.arange(128)[:, None]
    qc = np.arange(128)[None, :]
    tri_l = (kr <= qc).astype(np.float32)                # valid mask, left boundary
    tri_r = (qc <= kr).astype(np.float32)                # valid mask, right boundary
    return {
        "xt": np.ascontiguousarray(x.reshape(B * T, D).T.astype(bf)),
        "wq": np.ascontiguousarray(Wq[:, cols].astype(bf)),
        "wk": np.ascontiguousarray(Wk[:, cols].astype(bf)),
        "wv": np.ascontiguousarray(Wv[:, cols].astype(bf)),
        "wo": np.ascontiguousarray(Wo.astype(bf)),
        "cos_t": cos_t.astype(bf),
        "sin_s": sin_s.astype(bf),
        "tri_l": tri_l.astype(bf),
        "tri_r": tri_r.astype(bf),
    }


def host_assemble(results):
    """Concatenate the 8 per-core [512, 1024] chunks into [B, T, D]."""
    out = np.empty((B, T, D), np.float32)
    for c in range(N_CORES):
        out[c // 4, (c % 4) * TC:(c % 4 + 1) * TC, :] = results[c]["out"]
    return out


def build(nc, replicate=1, debug=False):
    x_d = nc.dram_tensor("xt", [D, B * T], BF16, kind="ExternalInput").ap()
    wq_d = nc.dram_tensor("wq", [D, HPC * DH], BF16, kind="ExternalInput").ap()
    wk_d = nc.dram_tensor("wk", [D, HPC * DH], BF16, kind="ExternalInput").ap()
    wv_d = nc.dram_tensor("wv", [D, HPC * DH], BF16, kind="ExternalInput").ap()
    wo_d = nc.dram_tensor("wo", [D, D], BF16, kind="ExternalInput").ap()
    cos_d = nc.dram_tensor("cos_t", [128, T], BF16, kind="ExternalInput").ap()
    sin_d = nc.dram_tensor("sin_s", [128, T], BF16, kind="ExternalInput").ap()
    tl_d = nc.dram_tensor("tri_l", [128, 128], BF16, kind="ExternalInput").ap()
    tr_d = nc.dram_tensor("tri_r", [128, 128], BF16, kind="ExternalInput").ap()
    out_d = nc.dram_tensor("out", [TC, D], F32, kind="ExternalOutput").ap()

    dbg = {}
    if debug:
        for name, shape, dt_ in [
            ("dbg_xt", [128, 8, 1024], BF16),
            ("dbg_q", [128, B, T], BF16),
            ("dbg_k", [128, B, T], BF16),
            ("dbg_vaug", [128, B * KT, 130], BF16),
            ("dbg_e", [128, ETOT], BF16),
            ("dbg_a", [65, T], BF16),
            ("dbg_at", [128, 8, TC], BF16),
            ("dbg_den", [16, TC], F32),
        ]:
            dbg[name] = nc.dram_tensor(name, shape, dt_, kind="ExternalOutput").ap()
    with tile.TileContext(nc) as tc:
        for _ in range(replicate):
            _build_once(nc, tc, x_d, wq_d, wk_d, wv_d, wo_d, cos_d, sin_d,
                        tl_d, tr_d, out_d, dbg)
    nc.compile()
    return nc


def _build_once(nc, tc, x_d, wq_d, wk_d, wv_d, wo_d, cos_d, sin_d, tl_d, tr_d,
                out_d, dbg={}):
    with tc.tile_pool(name="const", bufs=1) as constp, \
         tc.tile_pool(name="xt", bufs=2) as xtp, \
         tc.tile_pool(name="qk", bufs=1) as qkp, \
         tc.tile_pool(name="rope", bufs=2) as ropep, \
         tc.tile_pool(name="vt", bufs=2) as vtp, \
         tc.tile_pool(name="ebuf", bufs=2) as ep, \
         tc.tile_pool(name="abuf", bufs=1) as ap_, \
         tc.tile_pool(name="wop", bufs=1) as wop, \
         tc.tile_pool(name="fin", bufs=1) as finp, \
         tc.tile_pool(name="ps_small", bufs=2, space="PSUM") as pss, \
         tc.tile_pool(name="ps_big", bufs=2, space="PSUM") as psb, \
         tc.tile_pool(name="dram", bufs=1, space="DRAM") as dr:

        # ---------------- weights (bf16 from host) ----------------
        cos_sb = constp.tile([128, T], BF16, tag="cos")
        sin_sb = constp.tile([128, T], BF16, tag="sin")
        tl_sb = constp.tile([128, 128], BF16, tag="tl")
        tr_sb = constp.tile([128, 128], BF16, tag="tr")
        wq_sb = constp.tile([128, 8, HPC * DH], BF16, tag="wq")
        wk_sb = constp.tile([128, 8, HPC * DH], BF16, tag="wk")
        wv_sb = constp.tile([128, 8, HPC * DH], BF16, tag="wv")
        wo_sb = constp.tile([128, 8, D], BF16, tag="wo")
        # wq on the SP queue ahead of the first x quarter (so the first
        # projection matmuls start ASAP); wk/wv on the ACT queue; none of
        # them on Pool, which the gathers/staging keep busy.
        for cb in range(8):
            nc.sync.dma_start(wq_sb[:, cb, :], wq_d[cb * 128:(cb + 1) * 128, :])
        for cb in range(8):
            nc.scalar.dma_start(wk_sb[:, cb, :],
                                wk_d[cb * 128:(cb + 1) * 128, :])
            nc.scalar.dma_start(wv_sb[:, cb, :],
                                wv_d[cb * 128:(cb + 1) * 128, :])

        # ---------------- x^T (host pre-transposed bf16) ----------------
        NQ = 4                       # t-quarters of 1024 cols (over B*T=4096)
        QL = (B * T) // NQ           # 1024

        # Q^T/K^T/V^T projections, accumulating over c-blocks per quarter.
        # psum tile per (proj, t-chunk of 512): [128, 512]
        qraw = qkp.tile([128, B, T], BF16, tag="qp")     # becomes Q' after RoPE
        kraw = qkp.tile([128, B, T], BF16, tag="kp")
        v_aug = ap_.tile([128, B * KT, 130], BF16, tag="vaug")

        for q in range(NQ):
            xt = xtp.tile([128, 8, QL], BF16, tag="xt")
            for cb in range(8):
                nc.sync.dma_start(
                    xt[:, cb, :], x_d[cb * 128:(cb + 1) * 128,
                                      q * QL:(q + 1) * QL])
            for half in range(2):    # two 512-chunks per quarter
                t0 = q * QL + half * 512
                for pi, (wsb, dst) in enumerate(
                        ((wq_sb, qraw), (wk_sb, kraw))):
                    # alternate psum pools: ps_big is unused during projections
                    if (q * 6 + half * 3 + pi) % 2 == 0:
                        pt = pss.tile([128, 512], F32, tag="ps_small")
                    else:
                        pt = psb.tile([128, 512], F32, tag="ps_big",
                                      padded_shape=[128, MAXW])
                    for cb in range(8):
                        nc.tensor.matmul(
                            pt[:], wsb[:, cb, :],
                            xt[:, cb, half * 512:(half + 1) * 512],
                            start=(cb == 0), stop=(cb == 7))
                    b0, tt = divmod(t0, T)
                    nc.scalar.activation(dst[:, b0, tt:tt + 512], pt[:],
                                         AF.Copy)
                # V directly in natural layout (lhsT = x^T tile), written
                # straight into the 65-interleaved v_aug by DVE -- no DRAM
                # bounce, no transpose DMAs, no gather DMAs.
                for i in range(4):
                    ti = half * 4 + i
                    gb, gkt128 = divmod(t0 + i * 128, T)
                    gkt = gb * KT + gkt128 // 128
                    pv = pss.tile([128, 512], F32, tag="ps_small")
                    for cb in range(8):
                        nc.tensor.matmul(
                            pv[:, 0:128],
                            xt[:, cb, (half * 512 + i * 128) % 1024:
                               (half * 512 + i * 128) % 1024 + 128],
                            wv_sb[:, cb, :],
                            start=(cb == 0), stop=(cb == 7))
                    nc.vector.tensor_copy(v_aug[:, gkt, 0:64], pv[:, 0:64])
                    nc.vector.tensor_copy(v_aug[:, gkt, 65:129],
                                          pv[:, 64:128])

        nc.gpsimd.dma_start(cos_sb[:], cos_d[:])
        nc.gpsimd.dma_start(sin_sb[:], sin_d[:])
        nc.gpsimd.dma_start(tl_sb[:], tl_d[:])
        nc.gpsimd.dma_start(tr_sb[:], tr_d[:])
        for cb in range(8):
            nc.gpsimd.dma_start(wo_sb[:, cb, :], wo_d[cb * 128:(cb + 1) * 128, :])

        # ---------------- RoPE (on Q^T/K^T, per batch) ----------------
        for b in range(B):
            for dst in (qraw, kraw):
                sw = ropep.tile([128, T], BF16, tag="sw")
                # 32-block partition swap via SBUF->SBUF DMA
                for h2 in range(2):
                    p0 = h2 * 64
                    nc.sync.dma_start(sw[p0:p0 + 32, :],
                                      dst[p0 + 32:p0 + 64, b, :])
                    nc.sync.dma_start(sw[p0 + 32:p0 + 64, :],
                                      dst[p0:p0 + 32, b, :])
                nc.vector.tensor_mul(dst[:, b, :], dst[:, b, :], cos_sb[:])
                nc.vector.tensor_mul(sw[:], sw[:], sin_sb[:])
                nc.vector.tensor_add(dst[:, b, :], dst[:, b, :], sw[:])

        if "dbg_q" in dbg:
            nc.sync.dma_start(dbg["dbg_q"][:], qraw[:])
            nc.sync.dma_start(dbg["dbg_k"][:], kraw[:])

        one_view = v_aug[:].rearrange("p k (h e) -> p k h e", e=65)[:, :, :, 64]
        nc.vector.memset(one_view, 1.0)

        if "dbg_vaug" in dbg:
            nc.sync.dma_start(dbg["dbg_vaug"][:], v_aug[:])

        # ------------- all-to-all split by head-half (overlap) -------------
        # collective h2: chunk j -> core j = (batch j//4, t-chunk j%4);
        # rows 0-63 = A^T of the sender's head h2, row 64 = its denominator.
        a2a_in = [dr.tile([8, 65, TC], BF16, name=f"a2ai{i}", tag=f"a2ai{i}")
                  for i in range(2)]
        a2a_out = [dr.tile([8, 65, TC], BF16, name=f"a2ao{i}", tag=f"a2ao{i}")
                   for i in range(2)]
        at_sb = finp.tile([128, 8, TC], BF16, tag="at")
        den_sb = [finp.tile([8, TC], F32, name=f"den{i}", tag=f"den{i}")
                  for i in range(2)]
        rec_sb = [finp.tile([8, TC], F32, name=f"rec{i}", tag=f"rec{i}")
                  for i in range(2)]
        recs_sb = [vtp.tile([8, TC], F32, name=f"recs{i}", tag="rsc", bufs=3)
                   for i in range(2)]
        recb_sb = [finp.tile([8, TC], BF16, name=f"recb{i}", tag=f"recb{i}")
                   for i in range(2)]
        rec_dr = [dr.tile([8, TC], BF16, name=f"recdr{i}", tag=f"recdr{i}")
                  for i in range(2)]

        def emit_a2a(h2):
            for j in range(8):
                jb, jt = j // 4, j % 4
                cols = slice(jt * TC, (jt + 1) * TC)
                nc.gpsimd.dma_start(a2a_in[h2][j, 0:65, :],
                                  a_out[h2 * B + jb][0:65, cols])
            nc.gpsimd.collective_compute(
                "AllToAll", ALU.bypass, replica_groups=[list(range(N_CORES))],
                ins=[a2a_in[h2].opt()], outs=[a2a_out[h2].opt()])

        last_dve = {}

        def recv_a2a(h2):
            # den row c = head h2 of sender c; at rows 64*h2.. per c-block.
            # Every DVE op here is PINNED after the last attention-phase DVE
            # op: the scheduler otherwise hoists the reciprocal (which waits
            # on the collective-gated den DMA) into the middle of the h2=1
            # DVE stream, head-of-line blocking the queue for ~13 us.
            from concourse.tile_rust import add_dep_helper
            from concourse.dve_ops import RECIPROCAL_APPROX_NR
            anchor = last_dve[1].ins if 1 in last_dve else None

            def pin(bi):
                if anchor is not None:
                    add_dep_helper(bi.ins, anchor,
                                   sync=False,
                                   reason="keep recv chain behind attention "
                                          "DVE stream")
                return bi

            nc.gpsimd.dma_start(den_sb[h2][:], a2a_out[h2][:, 64, :])
            pin(nc.vector.reciprocal_approx_fast(out=recs_sb[h2][:],
                                                 in_=den_sb[h2][:]))
            pin(nc.vector._custom_dve(RECIPROCAL_APPROX_NR, out=rec_sb[h2][:],
                                      in0=den_sb[h2][:], in1=recs_sb[h2][:],
                                      s0=2.0))
            pin(nc.vector.tensor_copy(recb_sb[h2][:], rec_sb[h2][:]))
            nc.sync.dma_start(rec_dr[h2][:], recb_sb[h2][:])
            for c in range(8):
                nc.sync.dma_start(at_sb[64 * h2:64 * h2 + 64, c, :],
                                  a2a_out[h2][c, 0:64, :])
                r_sc = vtp.tile([128, TC], BF16, tag="rsc", bufs=3)
                p0 = 64 * h2
                nc.sync.dma_start(
                    r_sc[p0:p0 + 64, :],
                    rec_dr[h2][c:c + 1, :].to_broadcast((64, TC)))
                pin(nc.vector.tensor_mul(at_sb[p0:p0 + 64, c, :],
                                         at_sb[p0:p0 + 64, c, :],
                                         r_sc[p0:p0 + 64, :]))

        # ---------------- attention: head-major batch-paired phases --------
        a_out = [None] * 4   # [65, T] numerator^T (+den row 64) per (h2, b)
        for h2 in range(2):
            e_sb = {}
            for b in range(B):
                a_out[h2 * B + b] = ap_.tile([65, T], BF16, name=f"a{h2}{b}",
                                             tag=f"a{h2}{b}")
                e_sb[b] = ep.tile([128, ETOT], BF16, name=f"e{h2}{b}", tag="E")

            def emit_av(b, qc, h2=h2, e_sb=e_sb):
                q0 = qc * 512
                kts = [kt for kt in range(KT)
                       if window(kt * 128)[0] < q0 + 512
                       and window(kt * 128)[1] > q0]
                av = pss.tile([65, 512], F32, tag="ps_small")
                for i, kt in enumerate(kts):
                    ws, we = window(kt * 128)
                    lo = max(q0, ws)
                    hi = min(q0 + 512, we)
                    nc.tensor.matmul(
                        av[:, lo - q0:hi - q0],
                        v_aug[:, b * KT + kt, 65 * h2:65 * h2 + 65],
                        e_sb[b][:, EOFF[kt] + lo - ws:EOFF[kt] + hi - ws],
                        start=(i == 0), stop=(i == len(kts) - 1))
                last_dve[h2] = nc.vector.tensor_copy(
                    a_out[h2 * B + b][:, q0:q0 + 512], av[:])

            for kt in range(KT):
                k0 = kt * 128
                ws, we = window(k0)
                W = we - ws
                sc = {}
                for b in range(B):
                    # scores read Q'/K' straight out of qraw/kraw -- the old
                    # qd/kd rearrange DMAs bought nothing but latency
                    p0 = 64 * h2
                    sc[b] = psb.tile([128, MAXW], F32, name=f"sc{b}",
                                     tag="ps_big")
                    lhsT = kraw[p0:p0 + 64, b, k0:k0 + 128]
                    for s0 in range(0, W, 512):
                        s1 = min(s0 + 512, W)
                        nc.tensor.matmul(sc[b][:, s0:s1],
                                         lhsT,
                                         qraw[p0:p0 + 64, b, ws + s0:ws + s1],
                                         start=True, stop=True)
                eo = EOFF[kt]
                for b in range(B):
                    nc.scalar.activation(e_sb[b][:, eo:eo + W], sc[b][:, 0:W],
                                         AF.Exp, scale=SCALE)
                    if k0 >= WIN:
                        nc.vector.tensor_mul(e_sb[b][:, eo:eo + 128],
                                             e_sb[b][:, eo:eo + 128], tl_sb[:])
                    if k0 + 128 + WIN <= T:
                        nc.vector.tensor_mul(
                            e_sb[b][:, eo + W - 128:eo + W],
                            e_sb[b][:, eo + W - 128:eo + W], tr_sb[:])
                    for qc in range(4):
                        if min(4 * qc + 7, KT - 1) == kt:
                            emit_av(b, qc)
            if h2 == 0 and "dbg_e" in dbg:
                nc.sync.dma_start(dbg["dbg_e"][:], e_sb[0][:])
            emit_a2a(h2)

        if "dbg_a" in dbg:
            nc.sync.dma_start(dbg["dbg_a"][:], a_out[0][:])

        recv_a2a(0)
        recv_a2a(1)
        if "dbg_at" in dbg:
            nc.sync.dma_start(dbg["dbg_at"][:], at_sb[:])
            nc.sync.dma_start(dbg["dbg_den"][:], den_sb[:])
        # Wo split by head-half: the h2=0 half of the contraction only needs
        # the first collective, so it overlaps the second one; halves are
        # combined with an accumulating DMA into the output.
        for h2 in range(2):
            p0 = 64 * h2
            for tt in range(4):
                for mh in range(2):
                    po = pss.tile([128, 512], F32, tag="ps_small")
                    for c in range(8):
                        nc.tensor.matmul(
                            po[:],
                            at_sb[p0:p0 + 64, c, tt * 128:(tt + 1) * 128],
                            wo_sb[p0:p0 + 64, c, mh * 512:(mh + 1) * 512],
                            start=(c == 0), stop=(c == 7))
                    ot = vtp.tile([128, 512], F32, tag="ot")
                    nc.vector.tensor_copy(ot[:], po[:])
                    dst = out_d[tt * 128:(tt + 1) * 128,
                                mh * 512:(mh + 1) * 512]
                    if h2 == 0:
                        nc.sync.dma_start(dst, ot[:])
                    else:
                        nc.gpsimd.dma_start(dst, ot[:], accum_op=ALU.add)


# ---------------------------------------------------------------------------
# Self-contained entry point: kernel(**inputs) -> full output [2, 2048, 1024]
# ---------------------------------------------------------------------------
_CACHE = {}


def _get_nc():
    if "nc" in _CACHE:
        return _CACHE["nc"]
    import concourse.bacc as _bacc
    nc = _bacc.Bacc("TRN2", target_bir_lowering=False, debug=False,
                    num_devices=N_CORES)
    build(nc)
    _CACHE["nc"] = nc
    return nc


def kernel(x, Wq, Wk, Wv, Wo):
    from concourse.bass_utils import run_bass_kernel_spmd
    x, Wq, Wk, Wv, Wo = (np.asarray(a, np.float32) for a in (x, Wq, Wk, Wv, Wo))
    nc = _get_nc()
    in_maps = [host_inputs(x, Wq, Wk, Wv, Wo, c) for c in range(N_CORES)]
    res = run_bass_kernel_spmd(nc, in_maps, core_ids=list(range(N_CORES)))
    return host_assemble(res.results)



# revision 45
# speedup vs baseline: 1.0848x; 1.0134x over previous
"""Sliding-window attention kernel for 8 TRN2 NeuronCores.

Sharding: core c owns heads {2c, 2c+1} for BOTH batches (tensor parallel
over the 16 heads).  After attention, an all-to-all redistributes the
per-head outputs so core c owns output rows (batch c//4, t-chunk c%4),
where it applies the full Wo projection.

Per-core pipeline (all matmul compute in bf16, fp32 accumulation):
  1. x (both batches) f32->bf16 cast DMA, then xbar-transpose load -> x^T
  2. Q^T/K^T/V^T projections (PE), RoPE via partition-swap DMA + DVE
  3. V^T -> V (natural layout) via DRAM bounce + transpose DMA, augmented
     with a ones column per head (gives softmax denominator for free)
  4. per (head, batch): banded scores S^T = K'Q'^T (PE) -> exp (ACT) ->
     boundary-triangle masks (DVE) -> E^T in SBUF
  5. AV: out^T[d,q] accumulated over k-tiles (PE), denominator = row 64
  6. all-to-all (bf16 numerators + denominators)
  7. receiver: reciprocal of denominators, broadcast scale, Wo matmul
"""
import numpy as np
import ml_dtypes

import concourse.bass as bass
import concourse.bacc as bacc
import concourse.mybir as mybir
import concourse.tile as tile

F32 = mybir.dt.float32
BF16 = mybir.dt.bfloat16
AF = mybir.ActivationFunctionType
ALU = mybir.AluOpType

B, T, D = 2, 2048, 1024
H, DH = 16, 64
WIN = T // 4              # 512
N_CORES = 8
HPC = H // N_CORES        # heads per core = 2
TC = T // 4               # output t-chunk per core = 512
KT = T // 128             # k-tiles per (head,batch) = 16
SCALE = 1.0 / np.sqrt(DH)

NKT = 128                 # k-tile rows
MAXW = 1152               # max window width per k-tile


def window(k0):
    """q-window [ws, we) for k-tile starting at k0."""
    return max(k0 - WIN, 0), min(k0 + NKT + WIN, T)


EOFF = []
_off = 0
for _kt in range(KT):
    _ws, _we = max(_kt*128 - WIN, 0), min(_kt*128 + NKT + WIN, T)
    EOFF.append(_off)
    _off += _we - _ws
ETOT = _off


def host_inputs(x, Wq, Wk, Wv, Wo, core):
    """Build the per-core input map (host-side shard + constant tables)."""
    bf = ml_dtypes.bfloat16
    cols = slice(core * HPC * DH, (core + 1) * HPC * DH)
    t = np.arange(T, dtype=np.float64)
    inv = 1.0 / (10000.0 ** (np.arange(0, DH, 2, dtype=np.float64) / DH))
    f = (t[:, None] * inv[None, :]).astype(np.float32)   # [T, 32]
    cos1 = np.cos(f).astype(np.float32)                  # [T, 32]
    sin1 = np.sin(f).astype(np.float32)
    # ^T layout tables [128, T]: row r -> head-dim d = r % 64
    d = np.arange(128) % 64
    cos_t = cos1.T[d % 32]                               # [128, T]
    sin_t = sin1.T[d % 32]
    sgn = np.where(d < 32, -1.0, 1.0).astype(np.float32)[:, None]
    sin_s = sin_t * sgn                                  # signed sin for swap trick
    kr = np.arange(128)[:, None]
    qc = np.arange(128)[None, :]
    tri_l = (kr <= qc).astype(np.float32)                # valid mask, left boundary
    tri_r = (qc <= kr).astype(np.float32)                # valid mask, right boundary
    return {
        "xt": np.ascontiguousarray(x.reshape(B * T, D).T.astype(bf)),
        "wq": np.ascontiguousarray(Wq[:, cols].astype(bf)),
        "wk": np.ascontiguousarray(Wk[:, cols].astype(bf)),
        "wv": np.ascontiguousarray(Wv[:, cols].astype(bf)),
        "wo": np.ascontiguousarray(Wo.astype(bf)),
        "cos_t": cos_t.astype(bf),
        "sin_s": sin_s.astype(bf),
        "tri_l": tri_l.astype(bf),
        "tri_r": tri_r.astype(bf),
    }


def host_assemble(results):
    """Concatenate the 8 per-core [512, 1024] chunks into [B, T, D]."""
    out = np.empty((B, T, D), np.float32)
    for c in range(N_CORES):
        out[c // 4, (c % 4) * TC:(c % 4 + 1) * TC, :] = results[c]["out"]
    return out


def build(nc, replicate=1, debug=False):
    x_d = nc.dram_tensor("xt", [D, B * T], BF16, kind="ExternalInput").ap()
    wq_d = nc.dram_tensor("wq", [D, HPC * DH], BF16, kind="ExternalInput").ap()
    wk_d = nc.dram_tensor("wk", [D, HPC * DH], BF16, kind="ExternalInput").ap()
    wv_d = nc.dram_tensor("wv", [D, HPC * DH], BF16, kind="ExternalInput").ap()
    wo_d = nc.dram_tensor("wo", [D, D], BF16, kind="ExternalInput").ap()
    cos_d = nc.dram_tensor("cos_t", [128, T], BF16, kind="ExternalInput").ap()
    sin_d = nc.dram_tensor("sin_s", [128, T], BF16, kind="ExternalInput").ap()
    tl_d = nc.dram_tensor("tri_l", [128, 128], BF16, kind="ExternalInput").ap()
    tr_d = nc.dram_tensor("tri_r", [128, 128], BF16, kind="ExternalInput").ap()
    out_d = nc.dram_tensor("out", [TC, D], F32, kind="ExternalOutput").ap()

    dbg = {}
    if debug:
        for name, shape, dt_ in [
            ("dbg_xt", [128, 8, 1024], BF16),
            ("dbg_q", [128, B, T], BF16),
            ("dbg_k", [128, B, T], BF16),
            ("dbg_vaug", [128, B * KT, 130], BF16),
            ("dbg_e", [128, ETOT], BF16),
            ("dbg_a", [65, T], BF16),
            ("dbg_at", [128, 8, TC], BF16),
            ("dbg_den", [16, TC], F32),
        ]:
            dbg[name] = nc.dram_tensor(name, shape, dt_, kind="ExternalOutput").ap()
    with tile.TileContext(nc) as tc:
        for _ in range(replicate):
            _build_once(nc, tc, x_d, wq_d, wk_d, wv_d, wo_d, cos_d, sin_d,
                        tl_d, tr_d, out_d, dbg)
    nc.compile()
    return nc


def _build_once(nc, tc, x_d, wq_d, wk_d, wv_d, wo_d, cos_d, sin_d, tl_d, tr_d,
                out_d, dbg={}):
    with tc.tile_pool(name="const", bufs=1) as constp, \
         tc.tile_pool(name="xt", bufs=2) as xtp, \
         tc.tile_pool(name="qk", bufs=1) as qkp, \
         tc.tile_pool(name="rope", bufs=2) as ropep, \
         tc.tile_pool(name="vt", bufs=2) as vtp, \
         tc.tile_pool(name="ebuf", bufs=2) as ep, \
         tc.tile_pool(name="abuf", bufs=1) as ap_, \
         tc.tile_pool(name="wop", bufs=1) as wop, \
         tc.tile_pool(name="fin", bufs=1) as finp, \
         tc.tile_pool(name="ps_small", bufs=2, space="PSUM") as pss, \
         tc.tile_pool(name="ps_big", bufs=2, space="PSUM") as psb, \
         tc.tile_pool(name="dram", bufs=1, space="DRAM") as dr:

        # ---------------- weights (bf16 from host) ----------------
        cos_sb = constp.tile([128, T], BF16, tag="cos")
        sin_sb = constp.tile([128, T], BF16, tag="sin")
        tl_sb = constp.tile([128, 128], BF16, tag="tl")
        tr_sb = constp.tile([128, 128], BF16, tag="tr")
        wq_sb = constp.tile([128, 8, HPC * DH], BF16, tag="wq")
        wk_sb = constp.tile([128, 8, HPC * DH], BF16, tag="wk")
        wv_sb = constp.tile([128, 8, HPC * DH], BF16, tag="wv")
        wo_sb = constp.tile([128, 8, D], BF16, tag="wo")
        # wq on the SP queue ahead of the first x quarter (so the first
        # projection matmuls start ASAP); wk/wv on the ACT queue; none of
        # them on Pool, which the gathers/staging keep busy.
        for cb in range(8):
            nc.sync.dma_start(wq_sb[:, cb, :], wq_d[cb * 128:(cb + 1) * 128, :])
        for cb in range(8):
            nc.scalar.dma_start(wk_sb[:, cb, :],
                                wk_d[cb * 128:(cb + 1) * 128, :])
            nc.scalar.dma_start(wv_sb[:, cb, :],
                                wv_d[cb * 128:(cb + 1) * 128, :])

        # ---------------- x^T (host pre-transposed bf16) ----------------
        NQ = 4                       # t-quarters of 1024 cols (over B*T=4096)
        QL = (B * T) // NQ           # 1024

        # Q^T/K^T/V^T projections, accumulating over c-blocks per quarter.
        # psum tile per (proj, t-chunk of 512): [128, 512]
        qraw = qkp.tile([128, B, T], BF16, tag="qp")     # becomes Q' after RoPE
        kraw = qkp.tile([128, B, T], BF16, tag="kp")
        v_aug = ap_.tile([128, B * KT, 130], BF16, tag="vaug")

        for q in range(NQ):
            xt = xtp.tile([128, 8, QL], BF16, tag="xt")
            for cb in range(8):
                nc.sync.dma_start(
                    xt[:, cb, :], x_d[cb * 128:(cb + 1) * 128,
                                      q * QL:(q + 1) * QL])
            for half in range(2):    # two 512-chunks per quarter
                t0 = q * QL + half * 512
                for pi, (wsb, dst) in enumerate(
                        ((wq_sb, qraw), (wk_sb, kraw))):
                    # alternate psum pools: ps_big is unused during projections
                    if (q * 6 + half * 3 + pi) % 2 == 0:
                        pt = pss.tile([128, 512], F32, tag="ps_small")
                    else:
                        pt = psb.tile([128, 512], F32, tag="ps_big",
                                      padded_shape=[128, MAXW])
                    for cb in range(8):
                        nc.tensor.matmul(
                            pt[:], wsb[:, cb, :],
                            xt[:, cb, half * 512:(half + 1) * 512],
                            start=(cb == 0), stop=(cb == 7))
                    b0, tt = divmod(t0, T)
                    nc.scalar.activation(dst[:, b0, tt:tt + 512], pt[:],
                                         AF.Copy)
                # V directly in natural layout (lhsT = x^T tile), written
                # straight into the 65-interleaved v_aug by DVE -- no DRAM
                # bounce, no transpose DMAs, no gather DMAs.
                for i in range(4):
                    ti = half * 4 + i
                    gb, gkt128 = divmod(t0 + i * 128, T)
                    gkt = gb * KT + gkt128 // 128
                    pv = pss.tile([128, 512], F32, tag="ps_small")
                    for cb in range(8):
                        nc.tensor.matmul(
                            pv[:, 0:128],
                            xt[:, cb, (half * 512 + i * 128) % 1024:
                               (half * 512 + i * 128) % 1024 + 128],
                            wv_sb[:, cb, :],
                            start=(cb == 0), stop=(cb == 7))
                    nc.vector.tensor_copy(v_aug[:, gkt, 0:64], pv[:, 0:64])
                    nc.vector.tensor_copy(v_aug[:, gkt, 65:129],
                                          pv[:, 64:128])

        nc.gpsimd.dma_start(cos_sb[:], cos_d[:])
        nc.gpsimd.dma_start(sin_sb[:], sin_d[:])
        nc.gpsimd.dma_start(tl_sb[:], tl_d[:])
        nc.gpsimd.dma_start(tr_sb[:], tr_d[:])
        for cb in range(8):
            nc.gpsimd.dma_start(wo_sb[:, cb, :], wo_d[cb * 128:(cb + 1) * 128, :])

        # ---------------- RoPE (on Q^T/K^T, per batch) ----------------
        for b in range(B):
            for dst in (qraw, kraw):
                sw = ropep.tile([128, T], BF16, tag="sw")
                # 32-block partition swap via SBUF->SBUF DMA
                for h2 in range(2):
                    p0 = h2 * 64
                    nc.sync.dma_start(sw[p0:p0 + 32, :],
                                      dst[p0 + 32:p0 + 64, b, :])
                    nc.sync.dma_start(sw[p0 + 32:p0 + 64, :],
                                      dst[p0:p0 + 32, b, :])
                nc.vector.tensor_mul(dst[:, b, :], dst[:, b, :], cos_sb[:])
                nc.vector.tensor_mul(sw[:], sw[:], sin_sb[:])
                nc.vector.tensor_add(dst[:, b, :], dst[:, b, :], sw[:])

        if "dbg_q" in dbg:
            nc.sync.dma_start(dbg["dbg_q"][:], qraw[:])
            nc.sync.dma_start(dbg["dbg_k"][:], kraw[:])

        one_view = v_aug[:].rearrange("p k (h e) -> p k h e", e=65)[:, :, :, 64]
        nc.vector.memset(one_view, 1.0)

        if "dbg_vaug" in dbg:
            nc.sync.dma_start(dbg["dbg_vaug"][:], v_aug[:])

        # ------------- all-to-all split by head-half (overlap) -------------
        # collective h2: chunk j -> core j = (batch j//4, t-chunk j%4);
        # rows 0-63 = A^T of the sender's head h2, row 64 = its denominator.
        a2a_in = [dr.tile([8, 65, TC], BF16, name=f"a2ai{i}", tag=f"a2ai{i}")
                  for i in range(2)]
        a2a_out = [dr.tile([8, 65, TC], BF16, name=f"a2ao{i}", tag=f"a2ao{i}")
                   for i in range(2)]
        at_sb = finp.tile([128, 8, TC], BF16, tag="at")
        den_sb = [finp.tile([8, TC], F32, name=f"den{i}", tag=f"den{i}")
                  for i in range(2)]
        rec_sb = [finp.tile([8, TC], F32, name=f"rec{i}", tag=f"rec{i}")
                  for i in range(2)]
        recs_sb = [vtp.tile([8, TC], F32, name=f"recs{i}", tag="rsc", bufs=3)
                   for i in range(2)]
        recb_sb = [finp.tile([8, TC], BF16, name=f"recb{i}", tag=f"recb{i}")
                   for i in range(2)]
        rec_dr = [dr.tile([8, TC], BF16, name=f"recdr{i}", tag=f"recdr{i}")
                  for i in range(2)]

        def emit_a2a(h2):
            for j in range(8):
                jb, jt = j // 4, j % 4
                cols = slice(jt * TC, (jt + 1) * TC)
                nc.gpsimd.dma_start(a2a_in[h2][j, 0:65, :],
                                  a_out[h2 * B + jb][0:65, cols])
            nc.gpsimd.collective_compute(
                "AllToAll", ALU.bypass, replica_groups=[list(range(N_CORES))],
                ins=[a2a_in[h2].opt()], outs=[a2a_out[h2].opt()])

        last_dve = {}

        def recv_a2a(h2):
            # den row c = head h2 of sender c; at rows 64*h2.. per c-block.
            # Every DVE op here is PINNED after the last attention-phase DVE
            # op: the scheduler otherwise hoists the reciprocal (which waits
            # on the collective-gated den DMA) into the middle of the h2=1
            # DVE stream, head-of-line blocking the queue for ~13 us.
            from concourse.tile_rust import add_dep_helper
            from concourse.dve_ops import RECIPROCAL_APPROX_NR
            anchor = last_dve[1].ins if 1 in last_dve else None

            def pin(bi):
                if anchor is not None:
                    add_dep_helper(bi.ins, anchor,
                                   sync=False,
                                   reason="keep recv chain behind attention "
                                          "DVE stream")
                return bi

            nc.gpsimd.dma_start(den_sb[h2][:], a2a_out[h2][:, 64, :])
            pin(nc.vector.reciprocal_approx_fast(out=recs_sb[h2][:],
                                                 in_=den_sb[h2][:]))
            pin(nc.vector._custom_dve(RECIPROCAL_APPROX_NR, out=rec_sb[h2][:],
                                      in0=den_sb[h2][:], in1=recs_sb[h2][:],
                                      s0=2.0))
            pin(nc.vector.tensor_copy(recb_sb[h2][:], rec_sb[h2][:]))
            nc.sync.dma_start(rec_dr[h2][:], recb_sb[h2][:])
            for c in range(8):
                nc.sync.dma_start(at_sb[64 * h2:64 * h2 + 64, c, :],
                                  a2a_out[h2][c, 0:64, :])
                r_sc = vtp.tile([128, TC], BF16, tag="rsc", bufs=3)
                p0 = 64 * h2
                nc.sync.dma_start(
                    r_sc[p0:p0 + 64, :],
                    rec_dr[h2][c:c + 1, :].to_broadcast((64, TC)))
                pin(nc.vector.tensor_mul(at_sb[p0:p0 + 64, c, :],
                                         at_sb[p0:p0 + 64, c, :],
                                         r_sc[p0:p0 + 64, :]))

        # ---------------- attention: head-major batch-paired phases --------
        a_out = [None] * 4   # [65, T] numerator^T (+den row 64) per (h2, b)
        for h2 in range(2):
            e_sb = {}
            for b in range(B):
                a_out[h2 * B + b] = ap_.tile([65, T], BF16, name=f"a{h2}{b}",
                                             tag=f"a{h2}{b}")
                e_sb[b] = ep.tile([128, ETOT], BF16, name=f"e{h2}{b}", tag="E")

            def emit_av(b, qc, h2=h2, e_sb=e_sb):
                q0 = qc * 512
                kts = [kt for kt in range(KT)
                       if window(kt * 128)[0] < q0 + 512
                       and window(kt * 128)[1] > q0]
                av = pss.tile([65, 512], F32, tag="ps_small")
                for i, kt in enumerate(kts):
                    ws, we = window(kt * 128)
                    lo = max(q0, ws)
                    hi = min(q0 + 512, we)
                    nc.tensor.matmul(
                        av[:, lo - q0:hi - q0],
                        v_aug[:, b * KT + kt, 65 * h2:65 * h2 + 65],
                        e_sb[b][:, EOFF[kt] + lo - ws:EOFF[kt] + hi - ws],
                        start=(i == 0), stop=(i == len(kts) - 1))
                last_dve[h2] = nc.vector.tensor_copy(
                    a_out[h2 * B + b][:, q0:q0 + 512], av[:])

            for kt in range(KT):
                k0 = kt * 128
                ws, we = window(k0)
                W = we - ws
                sc = {}
                for b in range(B):
                    # scores read Q'/K' straight out of qraw/kraw -- the old
                    # qd/kd rearrange DMAs bought nothing but latency
                    p0 = 64 * h2
                    sc[b] = psb.tile([128, MAXW], F32, name=f"sc{b}",
                                     tag="ps_big")
                    lhsT = kraw[p0:p0 + 64, b, k0:k0 + 128]
                    for s0 in range(0, W, 512):
                        s1 = min(s0 + 512, W)
                        nc.tensor.matmul(sc[b][:, s0:s1],
                                         lhsT,
                                         qraw[p0:p0 + 64, b, ws + s0:ws + s1],
                                         start=True, stop=True)
                eo = EOFF[kt]
                for b in range(B):
                    nc.scalar.activation(e_sb[b][:, eo:eo + W], sc[b][:, 0:W],
                                         AF.Exp, scale=SCALE)
                    if k0 >= WIN:
                        nc.vector.tensor_mul(e_sb[b][:, eo:eo + 128],
                                             e_sb[b][:, eo:eo + 128], tl_sb[:])
                    if k0 + 128 + WIN <= T:
                        nc.vector.tensor_mul(
                            e_sb[b][:, eo + W - 128:eo + W],
                            e_sb[b][:, eo + W - 128:eo + W], tr_sb[:])
                    for qc in range(4):
                        if min(4 * qc + 7, KT - 1) == kt:
                            emit_av(b, qc)
            if h2 == 0 and "dbg_e" in dbg:
                nc.sync.dma_start(dbg["dbg_e"][:], e_sb[0][:])
            emit_a2a(h2)

        if "dbg_a" in dbg:
            nc.sync.dma_start(dbg["dbg_a"][:], a_out[0][:])

        recv_a2a(0)
        recv_a2a(1)
        if "dbg_at" in dbg:
            nc.sync.dma_start(dbg["dbg_at"][:], at_sb[:])
            nc.sync.dma_start(dbg["dbg_den"][:], den_sb[:])
        # Wo in a single K=128 pass: both halves have arrived once the
        # second collective lands, and one pass halves the matmul count and
        # replaces the serialized gpsimd accumulate-DMAs with plain stores.
        for tt in range(4):
            for mh in range(2):
                po = pss.tile([128, 512], F32, tag="ps_small")
                for c in range(8):
                    nc.tensor.matmul(
                        po[:],
                        at_sb[:, c, tt * 128:(tt + 1) * 128],
                        wo_sb[:, c, mh * 512:(mh + 1) * 512],
                        start=(c == 0), stop=(c == 7))
                ot = vtp.tile([128, 512], F32, tag="ot")
                nc.vector.tensor_copy(ot[:], po[:])
                nc.sync.dma_start(out_d[tt * 128:(tt + 1) * 128,
                                        mh * 512:(mh + 1) * 512], ot[:])


# ---------------------------------------------------------------------------
# Self-contained entry point: kernel(**inputs) -> full output [2, 2048, 1024]
# ---------------------------------------------------------------------------
_CACHE = {}


def _get_nc():
    if "nc" in _CACHE:
        return _CACHE["nc"]
    import concourse.bacc as _bacc
    nc = _bacc.Bacc("TRN2", target_bir_lowering=False, debug=False,
                    num_devices=N_CORES)
    build(nc)
    _CACHE["nc"] = nc
    return nc


def kernel(x, Wq, Wk, Wv, Wo):
    from concourse.bass_utils import run_bass_kernel_spmd
    x, Wq, Wk, Wv, Wo = (np.asarray(a, np.float32) for a in (x, Wq, Wk, Wv, Wo))
    nc = _get_nc()
    in_maps = [host_inputs(x, Wq, Wk, Wv, Wo, c) for c in range(N_CORES)]
    res = run_bass_kernel_spmd(nc, in_maps, core_ids=list(range(N_CORES)))
    return host_assemble(res.results)

